# revision 33
# baseline (speedup 1.0000x reference)
"""Trainium2 Bass kernel for MiniAttention (sparse_attention variant).

Reference computation (per batch b):
  qkv = x @ w_qkv -> split q,k,v  [H=12 heads, N=197 tokens, D=64]
  value_map = softmax((v @ v^T) * scale)          [H,N,N]   (output 4)
  scores    = (q*scale) @ k^T                     [H,N,N]   (output 2)
  attn_l    = conv_l mixing over heads of scores
  probs     = softmax(attn_l)                     [H,N,N]   (output 3)
  attn_w    = conv_w mixing over heads of probs
  out       = attn_w @ v -> proj -> + bias        [N,C]     (output 1)

Sharding: pure data-parallel over batch B=32 across 8 NeuronCores (4 each).

Layout trick: the head-mixing 1x1 convs contract over H=12, too small for the
128x128 PE.  Attention maps live as tiles [120 partitions = (h in 12) x
(j in 10 tokens), m free], so conv_l is one K=120 matmul with a constant
block-structured matrix WL[(h,j),(o,j')] = w_conv_l[o,h] d_jj'.  The scores
matmul is emitted directly into this layout using block-diagonal lhsT tiles
holding q for two heads per 128-row contraction chunk.  conv_w is fused with
the transpose the attn_w @ v matmul needs: awT[m,(h,j)] = probs^T @ WW2 with
WW2[(o,j),(h,j')] = w_conv_w[h,o] d_jj' as the *moving* operand, so attn_w is
produced directly m-major.

Softmax normalization (the divide) is done on the host: the kernel ships
exp() maps in bf16 plus the per-row sums/reciprocals in f32; conv_w's use of
normalized probs is handled by scaling WW2's rows with the reciprocals
(one [120,120] tensor_scalar per group).
"""

import numpy as np
import ml_dtypes
from contextlib import ExitStack

import concourse.bass as bass
import concourse.mybir as mybir
import concourse.tile as tile
from concourse import bacc
from concourse.bass_utils import run_bass_kernel_spmd

F32 = mybir.dt.float32
BF16 = mybir.dt.bfloat16
AF = mybir.ActivationFunctionType

B, N, C, H = 32, 197, 768, 12
D = C // H            # 64
SCALE = D ** -0.5     # 0.125
NCORES = 8
BPC = B // NCORES     # 4 batches per core
GS = 10               # token-group size in the (h, j) partition layout
G = 20                # number of token groups (covers NPAD=200 >= N)
NPAD = G * GS         # 200
HP = H * GS           # 120 partitions used in map tiles
KC = C // 128         # 6 contraction chunks of 128
MC = 3 * C // 128     # 18 output chunks of qkv
QW = 256              # padded qkvT tile width (zeros beyond N)
N0 = 128              # first token chunk
N1 = N - N0           # 69

_CACHE = {}


def _build_program():
    """Build the (SPMD, value-independent) Bass program once."""
    nc = bacc.Bacc(
        "TRN2", target_bir_lowering=False, debug=False, num_devices=NCORES
    )

    # ---- DRAM I/O -------------------------------------------------------
    # x arrives pre-transposed and bf16 from the host: [BPC, C, N]
    xT_d = nc.dram_tensor("xT", [BPC, C, N], BF16, kind="ExternalInput")
    wqkv_d = nc.dram_tensor("w_qkv", [C, 3 * C], BF16, kind="ExternalInput")
    wproj_d = nc.dram_tensor("w_proj", [C, C], BF16, kind="ExternalInput")
    bias_d = nc.dram_tensor("bias", [128, C], F32, kind="ExternalInput")
    wl_d = nc.dram_tensor("wl", [128, 128], BF16, kind="ExternalInput")
    ww2_d = nc.dram_tensor("ww2", [128, HP], BF16, kind="ExternalInput")
    idb_d = nc.dram_tensor("idb", [128, 128], BF16, kind="ExternalInput")

    out_d = nc.dram_tensor("attn_out", [BPC, N, C], F32, kind="ExternalOutput")
    scores_d = nc.dram_tensor("scores", [BPC, HP, G * N], BF16,
                              kind="ExternalOutput")
    pexp_d = nc.dram_tensor("pexp", [BPC, HP, G * N], BF16,
                            kind="ExternalOutput")
    prcp_d = nc.dram_tensor("prcp", [BPC, HP, G], F32, kind="ExternalOutput")
    vexp_d = nc.dram_tensor("vexp", [BPC, H // 2, 128, 4, N], BF16, kind="ExternalOutput")
    vsum_d = nc.dram_tensor("vsum", [BPC, 128, 2 * H], F32, kind="ExternalOutput")

    with tile.TileContext(nc) as tc, ExitStack() as ctx:
        consts = ctx.enter_context(tc.tile_pool(name="consts", bufs=1))
        qkvp = ctx.enter_context(tc.tile_pool(name="qkvp", bufs=2))
        work = ctx.enter_context(tc.tile_pool(name="work", bufs=2))
        scp = ctx.enter_context(tc.tile_pool(name="scp", bufs=2))
        exw = ctx.enter_context(tc.tile_pool(name="exw", bufs=2))
        dmap = ctx.enter_context(tc.tile_pool(name="dmap", bufs=2))
        sums = ctx.enter_context(tc.tile_pool(name="sums", bufs=4))
        psA = ctx.enter_context(
            tc.tile_pool(name="psA", bufs=4, space=bass.MemorySpace.PSUM)
        )
        ps2 = ctx.enter_context(
            tc.tile_pool(name="ps2", bufs=2, space=bass.MemorySpace.PSUM)
        )

        # ---- constants (qkv weights first, 3-way queue split) ------------
        wqkv_sb = []
        for k in range(KC):
            t = consts.tile([128, 3 * C], BF16, tag=f"wqkv{k}", name=f"wqkv{k}")
            eng = (nc.sync, nc.scalar, nc.gpsimd)[k % 3]
            eng.dma_start(t[:], wqkv_d[k * 128:(k + 1) * 128, :])
            wqkv_sb.append(t)
        idb_sb = consts.tile([128, 128], BF16, tag="idb", name="idb")
        nc.sync.dma_start(idb_sb[:], idb_d[:])
        wl_sb = consts.tile([128, 128], BF16, tag="wl", name="wl")
        nc.sync.dma_start(wl_sb[:], wl_d[:])
        ww2_sb = consts.tile([128, HP], BF16, tag="ww2", name="ww2")
        nc.sync.dma_start(ww2_sb[:], ww2_d[:])
        wproj_sb = []
        for k in range(KC):
            t = consts.tile([128, C], BF16, tag=f"wproj{k}", name=f"wproj{k}")
            eng = nc.scalar if k % 2 == 0 else nc.sync
            eng.dma_start(t[:], wproj_d[k * 128:(k + 1) * 128, :])
            wproj_sb.append(t)
        bias_sb = consts.tile([128, C], F32, tag="bias", name="bias")
        nc.scalar.dma_start(bias_sb[:], bias_d[:])
        zbias = consts.tile([128, 1], F32, tag="zbias", name="zbias")
        nc.gpsimd.memset(zbias[:], 0.0)

        # block-diagonal q lhsT tiles: allocated once, zeroed once; the
        # nonzero q blocks are fully overwritten every batch.  Per-group
        # stride is 128 columns (120 used + 8 zero) so the weight loads see
        # full 128-column tiles (FWL); psum rows 120:128 are junk.
        BDW = 128
        bd = []
        for k in range(KC):
            t = consts.tile([128, G * BDW], BF16, tag=f"bd{k}", name=f"bdt{k}")
            nc.gpsimd.memset(t[:], 0.0)
            bd.append(t)

        qkvT = {}   # batch -> list of 18 [128, NPAD] bf16 tile views

        def stage_qkv(b0):
            """Compute qkvT for batches b0, b0+1 (paired rhs)."""
            xT = []
            for k in range(KC):
                t = qkvp.tile([128, 2 * N], BF16, tag=f"xT{k}", name=f"xT{k}")
                for bi in range(2):
                    nc.sync.dma_start(
                        t[:, bi * N:(bi + 1) * N],
                        xT_d[b0 + bi, k * 128:(k + 1) * 128, :],
                    )
                xT.append(t)
            for mi in range(MC):
                pq = psA.tile([128, 2 * N], F32, tag="ps", name="ps_qkv")
                for k in range(KC):
                    nc.tensor.matmul(
                        pq[:],
                        wqkv_sb[k][:, mi * 128:(mi + 1) * 128],
                        xT[k][:],
                        start=(k == 0),
                        stop=(k == KC - 1),
                    )
                t = qkvp.tile([128, 2 * QW], BF16, tag=f"qkvT{mi}",
                              name=f"qkvT{mi}")
                t3 = t[:].rearrange("p (b n) -> p b n", n=QW)
                ceng = nc.vector if mi % 2 == 0 else nc.scalar
                if ceng is nc.vector:
                    nc.vector.tensor_copy(t3[:, :, :N], pq[:].rearrange(
                        "p (b n) -> p b n", n=N))
                else:
                    nc.scalar.copy(t3[:, :, :N], pq[:].rearrange(
                        "p (b n) -> p b n", n=N))
                nc.gpsimd.memset(t3[:, :, N:QW], 0.0)
                for bi in range(2):
                    qkvT.setdefault(b0 + bi, [None] * MC)[mi] = t[
                        :, bi * QW:bi * QW + QW
                    ]

        pending_tail = []
        for b in range(BPC):
            if b % 2 == 0:
                stage_qkv(b)
            qk = qkvT[b]

            # ---- fill block-diagonal q lhsT tiles for this batch ---------
            for k in range(KC):
                dst = bd[k][:].rearrange("p (g t) -> p g t", t=BDW)
                src = qk[k][:, :NPAD].rearrange("p (g j) -> p g j", j=GS)
                h0, h1 = 2 * k, 2 * k + 1
                nc.gpsimd.tensor_copy(
                    dst[0:64, :, h0 * GS:(h0 + 1) * GS], src[0:64, :, :]
                )
                nc.gpsimd.tensor_copy(
                    dst[64:128, :, h1 * GS:(h1 + 1) * GS], src[64:128, :, :]
                )

            # ---- v^T -> v (token-major) packed tiles ----------------------
            v0 = work.tile([128, H * D], BF16, tag="v0", name="v0")
            v1 = work.tile([128, H * D], BF16, tag="v1", name="v1")
            for h in range(H):
                ch = 2 * KC + h // 2
                r0 = (h % 2) * 64
                for (o, sz, vt) in ((0, N0, v0), (N0, N1, v1)):
                    pt = psA.tile([128, 128], BF16, tag="ps", name="ps_trb")
                    nc.tensor.transpose(
                        pt[:sz, :D],
                        qk[ch][r0:r0 + 64, o:o + sz],
                        idb_sb[r0:r0 + 64, r0:r0 + 64],
                    )
                    nc.vector.tensor_copy(
                        vt[:sz, h * D:(h + 1) * D], pt[:sz, :D]
                    )

            awT0 = work.tile([128, G * HP], BF16, tag="awT0", name="awT0")
            awT1 = work.tile([N1, G * HP], BF16, tag="awT1", name="awT1")

            # ---- scores (all groups first: dense PE run) ------------------
            sc_all = scp.tile([128, G * N], BF16, tag="sc_all", name="sc_all")
            sc_tiles = [sc_all[:, g * N:(g + 1) * N] for g in range(G)]
            for g in range(G):
                ps_sc = psA.tile([128, N], F32, tag="ps", name="ps_sc")
                for k in range(KC):
                    nc.tensor.matmul(
                        ps_sc[:],
                        bd[k][:, g * BDW:(g + 1) * BDW],
                        qk[KC + k][:, :N],
                        start=(k == 0),
                        stop=(k == KC - 1),
                    )
                eng = nc.vector if g % 2 == 0 else nc.scalar
                if eng is nc.vector:
                    nc.vector.tensor_copy(sc_tiles[g][:], ps_sc[:])
                else:
                    nc.scalar.copy(sc_tiles[g][:], ps_sc[:])
            nc.sync.dma_start(scores_d[b], sc_all[:HP, :])

            # ---- value map emitter -----------------------------------------
            vs_pack = dmap.tile([128, 2 * H], F32, tag="vs", name="vs")

            def emit_vmap_pair(hp, b=b, qk=qk, vs_pack=vs_pack):
                ch = 2 * KC + hp
                ps_v = ps2.tile([128, 1024], F32, tag="ps2", name="ps_v")
                for par in range(2):
                    r0 = par * 64
                    c0 = par * 512
                    nc.tensor.matmul(
                        ps_v[:, c0:c0 + N],
                        qk[ch][r0:r0 + 64, 0:N0],
                        qk[ch][r0:r0 + 64, :N],
                        start=True, stop=True,
                    )
                    nc.tensor.matmul(
                        ps_v[:, c0 + N:c0 + 2 * N],
                        qk[ch][r0:r0 + 64, N0:N0 + 128],
                        qk[ch][r0:r0 + 64, :N],
                        start=True, stop=True,
                    )
                pin = ps_v[:].rearrange("p (a q) -> p a q", a=2)[:, :, :2 * N]
                pin = pin.rearrange("p a (t m) -> p a t m", m=N)
                vex = exw.tile([128, 4 * N], BF16, tag="vex", name="vex")
                nc.scalar.activation(
                    vex[:].rearrange("p (a t m) -> p a t m", a=2, m=N),
                    pin, AF.Exp, bias=zbias[:], scale=float(SCALE),
                )
                nc.vector.tensor_reduce(
                    out=vs_pack[:, 4 * hp:4 * hp + 4],
                    in_=vex[:].rearrange("p (t m) -> p t m", m=N),
                    op=mybir.AluOpType.add,
                    axis=mybir.AxisListType.X,
                )
                nc.scalar.dma_start(
                    vexp_d[b, hp],
                    vex[:].rearrange("p (t m) -> p t m", m=N),
                )

            # ---- pass A: conv_l -> exp -> sums -> scaled WW2 ---------------
            # (vmap heads interleaved; results packed so pass B can run
            # densely after the deferred tail of the previous batch)
            rcp_pack = dmap.tile([128, G], F32, tag="rcp_pack", name="rcp_pack")
            nc.gpsimd.memset(rcp_pack[96:, :], 0.0)
            ex_all = exw.tile([128, G * N], BF16, tag="ex_all", name="ex_all")
            ww2s_all = exw.tile([128, G * HP], BF16, tag="ww2s_all",
                                name="ww2s_all")
            for t4 in range(G // 4):
                g0 = 4 * t4
                ps_al = ps2.tile([128, 1024], F32, tag="ps2", name="ps_al")
                for a in range(2):
                    nc.tensor.matmul(
                        ps_al[:, 512 * a:512 * a + 2 * N], wl_sb[:],
                        sc_all[:, (g0 + 2 * a) * N:(g0 + 2 * a + 2) * N],
                        start=True, stop=True,
                    )
                if t4 < G // 4 - 1:
                    emit_vmap_pair(t4)
                else:
                    emit_vmap_pair(4)
                    emit_vmap_pair(5)
                ex = ex_all[:, g0 * N:(g0 + 4) * N]
                pin = ps_al[:].rearrange("p (a q) -> p a q", a=2)[:, :, :2 * N]
                pin = pin.rearrange("p a (u m) -> p a u m", m=N)
                nc.scalar.activation(
                    ex.rearrange("p (a u m) -> p a u m", a=2, m=N),
                    pin, AF.Exp, bias=zbias[:, :],
                )
                sm4 = sums.tile([HP, 4], F32, tag="sm4", name="sm4")
                nc.vector.tensor_reduce(
                    out=sm4[:],
                    in_=ex[:HP].rearrange("p (u m) -> p u m", m=N),
                    op=mybir.AluOpType.add,
                    axis=mybir.AxisListType.X,
                )
                nc.vector.reciprocal(rcp_pack[:HP, g0:g0 + 4], sm4[:])
                for g in range(g0, g0 + 4):
                    nc.vector.tensor_scalar_mul(
                        ww2s_all[:, g * HP:(g + 1) * HP], ww2_sb[:],
                        rcp_pack[:, g:g + 1],
                    )  # pad rows: 0 * rcp(0) = 0
            nc.scalar.dma_start(pexp_d[b], ex_all[:HP, :])
            nc.sync.dma_start(prcp_d[b], rcp_pack[:HP, :])

            # ---- deferred tail of the previous batch -----------------------
            while pending_tail:
                pending_tail.pop(0)()

            # ---- pass B: conv_w^T (dense PE run) ---------------------------
            for g in range(G):
                for (o, sz, awt) in ((0, N0, awT0), (N0, N1, awT1)):
                    ps_aw = psA.tile([128, HP], F32, tag="ps", name="ps_aw")
                    nc.tensor.matmul(
                        ps_aw[:sz, :], ex_all[:, g * N + o:g * N + o + sz],
                        ww2s_all[:, g * HP:(g + 1) * HP],
                        start=True, stop=True,
                    )
                    if g % 5 < 3:
                        nc.vector.tensor_copy(
                            awt[:sz, g * HP:(g + 1) * HP], ps_aw[:sz, :]
                        )
                    else:
                        nc.scalar.copy(
                            awt[:sz, g * HP:(g + 1) * HP], ps_aw[:sz, :]
                        )
            nc.sync.dma_start(vsum_d[b, :, :], vs_pack[:])

            # ---- tail parts (deferred into the next batch's conv loop) ---
            pj_box = []

            def part_out_heads(hs, b=b, awT0=awT0, awT1=awT1, v0=v0, v1=v1):
                if not pj_box:
                    for cch in range(KC):
                        t = work.tile([128, N], BF16, tag=f"pj{cch}",
                                      name=f"pj{cch}")
                        pj_box.append(t)
                for h in hs:
                    ps_o = psA.tile([64, NPAD], F32, tag="ps", name="ps_o")
                    rhs0 = awT0[:].rearrange("p (g t) -> p g t", t=HP)
                    rhs1 = awT1[:].rearrange("p (g t) -> p g t", t=HP)
                    nc.tensor.matmul(
                        ps_o[:],
                        v0[:, h * D:(h + 1) * D],
                        rhs0[:, :, h * GS:(h + 1) * GS],
                        start=True,
                        stop=False,
                    )
                    nc.tensor.matmul(
                        ps_o[:],
                        v1[:N1, h * D:(h + 1) * D],
                        rhs1[:N1, :, h * GS:(h + 1) * GS],
                        start=False,
                        stop=True,
                    )
                    if h % 2 == 0:
                        nc.scalar.copy(
                            pj_box[h // 2][:64, :], ps_o[:, :N],
                        )
                    else:
                        nc.vector.tensor_copy(
                            pj_box[h // 2][64:128, :], ps_o[:, :N],
                        )

            def part_proj(o, sz, ci, f0, fsz, b=b):
                ps_p = psA.tile([128, 512], F32, tag="ps", name="ps_p")
                for k in range(KC):
                    nc.tensor.matmul(
                        ps_p[:sz, :fsz],
                        pj_box[k][:, o:o + sz],
                        wproj_sb[k][:, f0:f0 + fsz],
                        start=(k == 0),
                        stop=(k == KC - 1),
                    )
                ao = dmap.tile([128, 512], F32, tag=f"ao{ci}_{f0}",
                               name=f"ao{ci}_{f0}")
                nc.vector.tensor_tensor(
                    ao[:sz, :fsz], ps_p[:sz, :fsz], bias_sb[:sz, f0:f0 + fsz],
                    op=mybir.AluOpType.add,
                )
                nc.sync.dma_start(
                    out_d[b, o:o + sz, f0:f0 + fsz], ao[:sz, :fsz]
                )

            pending_tail.extend([
                lambda: part_out_heads(range(0, 3)),
                lambda: part_out_heads(range(3, 6)),
                lambda: part_out_heads(range(6, 9)),
                lambda: part_out_heads(range(9, 12)),
                lambda: part_proj(0, N0, 0, 0, 512),
                lambda: part_proj(0, N0, 0, 512, 256),
                lambda: part_proj(N0, N1, 1, 0, 512),
                lambda: part_proj(N0, N1, 1, 512, 256),
            ])

        while pending_tail:
            pending_tail.pop(0)()

    nc.compile()
    return nc


def _prep_inputs(x, w_qkv, w_proj, b_proj, w_conv_l, w_conv_w):
    x = np.asarray(x, dtype=np.float32)
    w_qkv = np.asarray(w_qkv, dtype=np.float32).copy()
    w_proj = np.asarray(w_proj, dtype=np.float32)
    b_proj = np.asarray(b_proj, dtype=np.float32)
    w_conv_l = np.asarray(w_conv_l, dtype=np.float32)
    w_conv_w = np.asarray(w_conv_w, dtype=np.float32)

    # fold the attention scale into the q columns of w_qkv
    w_qkv[:, :C] *= SCALE
    wqkv_bf = w_qkv.astype(ml_dtypes.bfloat16)
    wproj_bf = w_proj.astype(ml_dtypes.bfloat16)
    bias_rep = np.ascontiguousarray(np.broadcast_to(b_proj, (128, C)))

    # x transposed per batch, bf16: [B, C, N]
    xT = np.ascontiguousarray(
        x.transpose(0, 2, 1).astype(ml_dtypes.bfloat16)
    )

    # conv_l as lhsT: WL[(h,j), (o,j)] = w_conv_l[o, h]; padded to 128 cols
    wl_b = np.zeros((128, 128), dtype=np.float32)
    # conv_w as moving operand: WW2[(o,j), (h,j)] = w_conv_w[h, o]
    ww2_b = np.zeros((128, HP), dtype=np.float32)
    idx = np.arange(GS)
    for a in range(H):
        for o in range(H):
            wl_b[a * GS + idx, o * GS + idx] = w_conv_l[o, a]
            ww2_b[a * GS + idx, o * GS + idx] = w_conv_w[o, a]
    wl_b = wl_b.astype(ml_dtypes.bfloat16)
    ww2_b = ww2_b.astype(ml_dtypes.bfloat16)
    idb = np.eye(128, dtype=ml_dtypes.bfloat16)

    in_maps = []
    for c in range(NCORES):
        in_maps.append({
            "xT": np.ascontiguousarray(xT[c * BPC:(c + 1) * BPC]),
            "w_qkv": wqkv_bf,
            "w_proj": wproj_bf,
            "bias": bias_rep,
            "wl": wl_b,
            "ww2": ww2_b,
            "idb": idb,
        })
    return in_maps


def _postprocess(outs):
    """Gather per-core results, normalize softmaxes, upcast to f32."""
    att = np.concatenate([o["attn_out"] for o in outs], axis=0)

    def unpack_map(a):
        # [B, HP=(h,j), G*N=(g,m)] -> [B, H, N, N] with n = g*GS + j
        a = a.reshape(B, H, GS, G, N).transpose(0, 1, 3, 2, 4)
        return a.reshape(B, H, NPAD, N)[:, :, :N, :]

    sc = unpack_map(
        np.concatenate([o["scores"] for o in outs], axis=0)
    ).astype(np.float32)

    pexp = unpack_map(np.concatenate([o["pexp"] for o in outs], axis=0))
    prcp = np.concatenate([o["prcp"] for o in outs], axis=0)  # [B, HP, G]
    n_idx = np.arange(N)
    prcp = prcp.reshape(B, H, GS, G)
    rcp = prcp[:, :, n_idx % GS, n_idx // GS]  # [B, H, N]
    pr = pexp.astype(np.float32) * rcp[:, :, :, None]

    # vexp: [B, H/2, 128, 4=(par,chunk), N] -> [B, H, N, N]
    vexp_r = np.concatenate([o["vexp"] for o in outs], axis=0)
    vexp_r = vexp_r.reshape(B, H // 2, 128, 2, 2, N).transpose(0, 1, 3, 2, 4, 5)
    vexp_r = vexp_r.reshape(B, H, 128, 2, N)
    vexp = np.concatenate(
        [vexp_r[:, :, :, 0, :], vexp_r[:, :, :N1, 1, :]], axis=2
    )
    vsum = np.concatenate([o["vsum"] for o in outs], axis=0)  # [B, 128, 2H]
    vsum = vsum.reshape(B, 128, H // 2, 2, 2).transpose(0, 1, 2, 3, 4)
    vsum = vsum.reshape(B, 128, H, 2)  # [..., h=(hp,par), chunk]
    vs = np.concatenate(
        [vsum[:, :, :, 0], vsum[:, :N1, :, 1]], axis=1
    ).transpose(0, 2, 1)  # [B, H, N]
    vm = vexp.astype(np.float32) / vs[:, :, :, None]

    att = np.ascontiguousarray(att, dtype=np.float32)
    return att, sc, np.ascontiguousarray(pr), np.ascontiguousarray(vm)


def _ensure_trace_support():
    """Install the antenv.axon_hooks NTFF shim missing from this image."""
    import sys
    import types
    try:
        import antenv.axon_hooks  # noqa: F401
        return
    except ImportError:
        pass
    import antenv
    from trn_agent_boot.trn_boot import _ntff_profile_via_ctypes
    hook = {"fn": _ntff_profile_via_ctypes("/opt/axon/libaxon_pjrt.so")}
    mod = types.ModuleType("antenv.axon_hooks")
    mod.get_axon_ntff_profile_hook = lambda: hook["fn"]
    mod.set_axon_ntff_profile_hook = lambda fn: hook.update(fn=fn)
    sys.modules["antenv.axon_hooks"] = mod
    antenv.axon_hooks = mod
    import concourse.bass_utils as bu
    bu.upload_artifacts = lambda tmpdir: f"local://{tmpdir}"


def _run(inputs, trace=False, trace_kwargs=None):
    if trace:
        _ensure_trace_support()
    if "nc" not in _CACHE:
        _CACHE["nc"] = _build_program()
    nc = _CACHE["nc"]
    in_maps = _prep_inputs(**inputs)
    res = run_bass_kernel_spmd(
        nc, in_maps, list(range(NCORES)), trace=trace,
        **({"trace_kwargs": trace_kwargs} if trace_kwargs else {}),
    )
    return _postprocess(res.results), res


def kernel(**inputs):
    (att, sc, pr, vm), _ = _run(inputs, trace=False)
    return att, sc, pr, vm


# revision 34
# speedup vs baseline: 1.0488x; 1.0488x over previous
"""Trainium2 Bass kernel for MiniAttention (sparse_attention variant).

Reference computation (per batch b):
  qkv = x @ w_qkv -> split q,k,v  [H=12 heads, N=197 tokens, D=64]
  value_map = softmax((v @ v^T) * scale)          [H,N,N]   (output 4)
  scores    = (q*scale) @ k^T                     [H,N,N]   (output 2)
  attn_l    = conv_l mixing over heads of scores
  probs     = softmax(attn_l)                     [H,N,N]   (output 3)
  attn_w    = conv_w mixing over heads of probs
  out       = attn_w @ v -> proj -> + bias        [N,C]     (output 1)

Sharding: pure data-parallel over batch B=32 across 8 NeuronCores (4 each).

Layout trick: the head-mixing 1x1 convs contract over H=12, too small for the
128x128 PE.  Attention maps live as tiles [120 partitions = (h in 12) x
(j in 10 tokens), m free], so conv_l is one K=120 matmul with a constant
block-structured matrix WL[(h,j),(o,j')] = w_conv_l[o,h] d_jj'.  The scores
matmul is emitted directly into this layout using block-diagonal lhsT tiles
holding q for two heads per 128-row contraction chunk.  conv_w is fused with
the transpose the attn_w @ v matmul needs: awT[m,(h,j)] = probs^T @ WW2 with
WW2[(o,j),(h,j')] = w_conv_w[h,o] d_jj' as the *moving* operand, so attn_w is
produced directly m-major.

Softmax normalization (the divide) is done on the host: the kernel ships
exp() maps in bf16 plus the per-row sums/reciprocals in f32; conv_w's use of
normalized probs is handled by scaling WW2's rows with the reciprocals
(one [120,120] tensor_scalar per group).
"""

import numpy as np
import ml_dtypes
from contextlib import ExitStack

import concourse.bass as bass
import concourse.mybir as mybir
import concourse.tile as tile
from concourse import bacc
from concourse.bass_utils import run_bass_kernel_spmd

F32 = mybir.dt.float32
BF16 = mybir.dt.bfloat16
AF = mybir.ActivationFunctionType

B, N, C, H = 32, 197, 768, 12
D = C // H            # 64
SCALE = D ** -0.5     # 0.125
NCORES = 8
BPC = B // NCORES     # 4 batches per core
GS = 10               # token-group size in the (h, j) partition layout
G = 20                # number of token groups (covers NPAD=200 >= N)
NPAD = G * GS         # 200
HP = H * GS           # 120 partitions used in map tiles
KC = C // 128         # 6 contraction chunks of 128
MC = 3 * C // 128     # 18 output chunks of qkv
QW = 256              # padded qkvT tile width (zeros beyond N)
N0 = 128              # first token chunk
N1 = N - N0           # 69

_CACHE = {}


def _build_program():
    """Build the (SPMD, value-independent) Bass program once."""
    nc = bacc.Bacc(
        "TRN2", target_bir_lowering=False, debug=False, num_devices=NCORES
    )

    # ---- DRAM I/O -------------------------------------------------------
    # x arrives pre-transposed and bf16 from the host: [BPC, C, N]
    xT_d = nc.dram_tensor("xT", [BPC, C, N], BF16, kind="ExternalInput")
    wqkv_d = nc.dram_tensor("w_qkv", [C, 3 * C], BF16, kind="ExternalInput")
    wproj_d = nc.dram_tensor("w_proj", [C, C], BF16, kind="ExternalInput")
    bias_d = nc.dram_tensor("bias", [128, C], F32, kind="ExternalInput")
    wl_d = nc.dram_tensor("wl", [128, 128], BF16, kind="ExternalInput")
    ww2_d = nc.dram_tensor("ww2", [128, HP], BF16, kind="ExternalInput")
    idb_d = nc.dram_tensor("idb", [128, 128], BF16, kind="ExternalInput")

    out_d = nc.dram_tensor("attn_out", [BPC, N, C], F32, kind="ExternalOutput")
    scores_d = nc.dram_tensor("scores", [BPC, HP, G * N], BF16,
                              kind="ExternalOutput")
    pexp_d = nc.dram_tensor("pexp", [BPC, HP, G * N], BF16,
                            kind="ExternalOutput")
    prcp_d = nc.dram_tensor("prcp", [BPC, HP, G], F32, kind="ExternalOutput")
    vexp_d = nc.dram_tensor("vexp", [BPC, H // 2, 128, 4, N], BF16, kind="ExternalOutput")
    vsum_d = nc.dram_tensor("vsum", [BPC, 128, 2 * H], F32, kind="ExternalOutput")

    with tile.TileContext(nc) as tc, ExitStack() as ctx:
        consts = ctx.enter_context(tc.tile_pool(name="consts", bufs=1))
        qkvp = ctx.enter_context(tc.tile_pool(name="qkvp", bufs=2))
        work = ctx.enter_context(tc.tile_pool(name="work", bufs=2))
        scp = ctx.enter_context(tc.tile_pool(name="scp", bufs=2))
        exw = ctx.enter_context(tc.tile_pool(name="exw", bufs=2))
        dmap = ctx.enter_context(tc.tile_pool(name="dmap", bufs=2))
        sums = ctx.enter_context(tc.tile_pool(name="sums", bufs=4))
        psA = ctx.enter_context(
            tc.tile_pool(name="psA", bufs=4, space=bass.MemorySpace.PSUM)
        )
        ps2 = ctx.enter_context(
            tc.tile_pool(name="ps2", bufs=2, space=bass.MemorySpace.PSUM)
        )

        # ---- constants (qkv weights first, 3-way queue split) ------------
        wqkv_sb = []
        for k in range(KC):
            t = consts.tile([128, 3 * C], BF16, tag=f"wqkv{k}", name=f"wqkv{k}")
            eng = nc.sync if k % 2 == 0 else nc.scalar
            eng.dma_start(t[:], wqkv_d[k * 128:(k + 1) * 128, :])
            wqkv_sb.append(t)
        idb_sb = consts.tile([128, 128], BF16, tag="idb", name="idb")
        nc.sync.dma_start(idb_sb[:], idb_d[:])
        wl_sb = consts.tile([128, 128], BF16, tag="wl", name="wl")
        nc.sync.dma_start(wl_sb[:], wl_d[:])
        ww2_sb = consts.tile([128, HP], BF16, tag="ww2", name="ww2")
        nc.sync.dma_start(ww2_sb[:], ww2_d[:])
        wproj_sb = []
        for k in range(KC):
            t = consts.tile([128, C], BF16, tag=f"wproj{k}", name=f"wproj{k}")
            eng = nc.scalar if k % 2 == 0 else nc.sync
            eng.dma_start(t[:], wproj_d[k * 128:(k + 1) * 128, :])
            wproj_sb.append(t)
        bias_sb = consts.tile([128, C], F32, tag="bias", name="bias")
        nc.scalar.dma_start(bias_sb[:], bias_d[:])
        zbias = consts.tile([128, 1], F32, tag="zbias", name="zbias")
        nc.gpsimd.memset(zbias[:], 0.0)

        # block-diagonal q lhsT tiles: allocated once, zeroed once; the
        # nonzero q blocks are fully overwritten every batch.  Per-group
        # stride is 128 columns (120 used + 8 zero) so the weight loads see
        # full 128-column tiles (FWL); psum rows 120:128 are junk.
        BDW = 128
        bd = []
        for k in range(KC):
            t = consts.tile([128, G * BDW], BF16, tag=f"bd{k}", name=f"bdt{k}")
            nc.gpsimd.memset(t[:], 0.0)
            bd.append(t)

        qkvT = {}   # batch -> list of 18 [128, NPAD] bf16 tile views

        def stage_qkv(b0):
            """Compute qkvT for batches b0, b0+1 (paired rhs)."""
            xT = []
            for k in range(KC):
                t = qkvp.tile([128, 2 * N], BF16, tag=f"xT{k}", name=f"xT{k}")
                for bi in range(2):
                    nc.sync.dma_start(
                        t[:, bi * N:(bi + 1) * N],
                        xT_d[b0 + bi, k * 128:(k + 1) * 128, :],
                    )
                xT.append(t)
            for mi in range(MC):
                pq = psA.tile([128, 2 * N], F32, tag="ps", name="ps_qkv")
                for k in range(KC):
                    nc.tensor.matmul(
                        pq[:],
                        wqkv_sb[k][:, mi * 128:(mi + 1) * 128],
                        xT[k][:],
                        start=(k == 0),
                        stop=(k == KC - 1),
                    )
                t = qkvp.tile([128, 2 * QW], BF16, tag=f"qkvT{mi}",
                              name=f"qkvT{mi}")
                t3 = t[:].rearrange("p (b n) -> p b n", n=QW)
                ceng = nc.vector if mi % 2 == 0 else nc.scalar
                if ceng is nc.vector:
                    nc.vector.tensor_copy(t3[:, :, :N], pq[:].rearrange(
                        "p (b n) -> p b n", n=N))
                else:
                    nc.scalar.copy(t3[:, :, :N], pq[:].rearrange(
                        "p (b n) -> p b n", n=N))
                nc.gpsimd.memset(t3[:, :, N:QW], 0.0)
                for bi in range(2):
                    qkvT.setdefault(b0 + bi, [None] * MC)[mi] = t[
                        :, bi * QW:bi * QW + QW
                    ]

        pending_tail = []
        for b in range(BPC):
            if b % 2 == 0:
                stage_qkv(b)
            qk = qkvT[b]

            # ---- fill block-diagonal q lhsT tiles for this batch ---------
            for k in range(KC):
                dst = bd[k][:].rearrange("p (g t) -> p g t", t=BDW)
                src = qk[k][:, :NPAD].rearrange("p (g j) -> p g j", j=GS)
                h0, h1 = 2 * k, 2 * k + 1
                nc.gpsimd.tensor_copy(
                    dst[0:64, :, h0 * GS:(h0 + 1) * GS], src[0:64, :, :]
                )
                nc.gpsimd.tensor_copy(
                    dst[64:128, :, h1 * GS:(h1 + 1) * GS], src[64:128, :, :]
                )

            # ---- v^T -> v (token-major) packed tiles ----------------------
            v0 = work.tile([128, H * D], BF16, tag="v0", name="v0")
            v1 = work.tile([128, H * D], BF16, tag="v1", name="v1")
            for h in range(H):
                ch = 2 * KC + h // 2
                r0 = (h % 2) * 64
                for (o, sz, vt) in ((0, N0, v0), (N0, N1, v1)):
                    pt = psA.tile([128, 128], BF16, tag="ps", name="ps_trb")
                    nc.tensor.transpose(
                        pt[:sz, :D],
                        qk[ch][r0:r0 + 64, o:o + sz],
                        idb_sb[r0:r0 + 64, r0:r0 + 64],
                    )
                    nc.vector.tensor_copy(
                        vt[:sz, h * D:(h + 1) * D], pt[:sz, :D]
                    )

            awT0 = work.tile([128, G * HP], BF16, tag="awT0", name="awT0")
            awT1 = work.tile([N1, G * HP], BF16, tag="awT1", name="awT1")

            # ---- scores (all groups first: dense PE run) ------------------
            sc_all = scp.tile([128, G * N], BF16, tag="sc_all", name="sc_all")
            sc_tiles = [sc_all[:, g * N:(g + 1) * N] for g in range(G)]
            for g in range(G):
                ps_sc = psA.tile([128, N], F32, tag="ps", name="ps_sc")
                for k in range(KC):
                    nc.tensor.matmul(
                        ps_sc[:],
                        bd[k][:, g * BDW:(g + 1) * BDW],
                        qk[KC + k][:, :N],
                        start=(k == 0),
                        stop=(k == KC - 1),
                    )
                eng = nc.vector if g % 2 == 0 else nc.scalar
                if eng is nc.vector:
                    nc.vector.tensor_copy(sc_tiles[g][:], ps_sc[:])
                else:
                    nc.scalar.copy(sc_tiles[g][:], ps_sc[:])
            nc.sync.dma_start(scores_d[b], sc_all[:HP, :])

            # ---- value map emitter -----------------------------------------
            vs_pack = dmap.tile([128, 2 * H], F32, tag="vs", name="vs")

            def emit_vmap_pair(hp, b=b, qk=qk, vs_pack=vs_pack):
                ch = 2 * KC + hp
                ps_v = ps2.tile([128, 1024], F32, tag="ps2", name="ps_v")
                for par in range(2):
                    r0 = par * 64
                    c0 = par * 512
                    nc.tensor.matmul(
                        ps_v[:, c0:c0 + N],
                        qk[ch][r0:r0 + 64, 0:N0],
                        qk[ch][r0:r0 + 64, :N],
                        start=True, stop=True,
                    )
                    nc.tensor.matmul(
                        ps_v[:, c0 + N:c0 + 2 * N],
                        qk[ch][r0:r0 + 64, N0:N0 + 128],
                        qk[ch][r0:r0 + 64, :N],
                        start=True, stop=True,
                    )
                pin = ps_v[:].rearrange("p (a q) -> p a q", a=2)[:, :, :2 * N]
                pin = pin.rearrange("p a (t m) -> p a t m", m=N)
                vex = exw.tile([128, 4 * N], BF16, tag="vex", name="vex")
                nc.scalar.activation(
                    vex[:].rearrange("p (a t m) -> p a t m", a=2, m=N),
                    pin, AF.Exp, bias=zbias[:], scale=float(SCALE),
                )
                nc.vector.tensor_reduce(
                    out=vs_pack[:, 4 * hp:4 * hp + 4],
                    in_=vex[:].rearrange("p (t m) -> p t m", m=N),
                    op=mybir.AluOpType.add,
                    axis=mybir.AxisListType.X,
                )
                nc.sync.dma_start(
                    vexp_d[b, hp],
                    vex[:].rearrange("p (t m) -> p t m", m=N),
                )

            # ---- pass A: conv_l -> exp -> sums -> scaled WW2 ---------------
            # (vmap heads interleaved; results packed so pass B can run
            # densely after the deferred tail of the previous batch)
            rcp_pack = dmap.tile([128, G], F32, tag="rcp_pack", name="rcp_pack")
            nc.gpsimd.memset(rcp_pack[96:, :], 0.0)
            ex_all = exw.tile([128, G * N], BF16, tag="ex_all", name="ex_all")
            ww2s_all = exw.tile([128, G * HP], BF16, tag="ww2s_all",
                                name="ww2s_all")
            for t4 in range(G // 4):
                g0 = 4 * t4
                ps_al = ps2.tile([128, 1024], F32, tag="ps2", name="ps_al")
                for a in range(2):
                    nc.tensor.matmul(
                        ps_al[:, 512 * a:512 * a + 2 * N], wl_sb[:],
                        sc_all[:, (g0 + 2 * a) * N:(g0 + 2 * a + 2) * N],
                        start=True, stop=True,
                    )
                if t4 < G // 4 - 1:
                    emit_vmap_pair(t4)
                else:
                    emit_vmap_pair(4)
                    emit_vmap_pair(5)
                ex = ex_all[:, g0 * N:(g0 + 4) * N]
                pin = ps_al[:].rearrange("p (a q) -> p a q", a=2)[:, :, :2 * N]
                pin = pin.rearrange("p a (u m) -> p a u m", m=N)
                nc.scalar.activation(
                    ex.rearrange("p (a u m) -> p a u m", a=2, m=N),
                    pin, AF.Exp, bias=zbias[:, :],
                )
                sm4 = sums.tile([HP, 4], F32, tag="sm4", name="sm4")
                nc.vector.tensor_reduce(
                    out=sm4[:],
                    in_=ex[:HP].rearrange("p (u m) -> p u m", m=N),
                    op=mybir.AluOpType.add,
                    axis=mybir.AxisListType.X,
                )
                nc.vector.reciprocal(rcp_pack[:HP, g0:g0 + 4], sm4[:])
                for g in range(g0, g0 + 4):
                    nc.vector.tensor_scalar_mul(
                        ww2s_all[:, g * HP:(g + 1) * HP], ww2_sb[:],
                        rcp_pack[:, g:g + 1],
                    )  # pad rows: 0 * rcp(0) = 0
            nc.scalar.dma_start(pexp_d[b], ex_all[:HP, :])
            nc.sync.dma_start(prcp_d[b], rcp_pack[:HP, :])

            # ---- deferred tail of the previous batch -----------------------
            while pending_tail:
                pending_tail.pop(0)()

            # ---- pass B: conv_w^T (dense PE run) ---------------------------
            for g in range(G):
                for (o, sz, awt) in ((0, N0, awT0), (N0, N1, awT1)):
                    ps_aw = psA.tile([128, HP], F32, tag="ps", name="ps_aw")
                    nc.tensor.matmul(
                        ps_aw[:sz, :], ex_all[:, g * N + o:g * N + o + sz],
                        ww2s_all[:, g * HP:(g + 1) * HP],
                        start=True, stop=True,
                    )
                    if g % 5 < 3:
                        nc.vector.tensor_copy(
                            awt[:sz, g * HP:(g + 1) * HP], ps_aw[:sz, :]
                        )
                    else:
                        nc.scalar.copy(
                            awt[:sz, g * HP:(g + 1) * HP], ps_aw[:sz, :]
                        )
            nc.sync.dma_start(vsum_d[b, :, :], vs_pack[:])

            # ---- tail parts (deferred into the next batch's conv loop) ---
            pj_box = []

            def part_out_heads(hs, b=b, awT0=awT0, awT1=awT1, v0=v0, v1=v1):
                if not pj_box:
                    for cch in range(KC):
                        t = work.tile([128, N], BF16, tag=f"pj{cch}",
                                      name=f"pj{cch}")
                        pj_box.append(t)
                for h in hs:
                    ps_o = psA.tile([64, NPAD], F32, tag="ps", name="ps_o")
                    rhs0 = awT0[:].rearrange("p (g t) -> p g t", t=HP)
                    rhs1 = awT1[:].rearrange("p (g t) -> p g t", t=HP)
                    nc.tensor.matmul(
                        ps_o[:],
                        v0[:, h * D:(h + 1) * D],
                        rhs0[:, :, h * GS:(h + 1) * GS],
                        start=True,
                        stop=False,
                    )
                    nc.tensor.matmul(
                        ps_o[:],
                        v1[:N1, h * D:(h + 1) * D],
                        rhs1[:N1, :, h * GS:(h + 1) * GS],
                        start=False,
                        stop=True,
                    )
                    if h % 2 == 0:
                        nc.scalar.copy(
                            pj_box[h // 2][:64, :], ps_o[:, :N],
                        )
                    else:
                        nc.vector.tensor_copy(
                            pj_box[h // 2][64:128, :], ps_o[:, :N],
                        )

            def part_proj(o, sz, ci, f0, fsz, b=b):
                ps_p = psA.tile([128, 512], F32, tag="ps", name="ps_p")
                for k in range(KC):
                    nc.tensor.matmul(
                        ps_p[:sz, :fsz],
                        pj_box[k][:, o:o + sz],
                        wproj_sb[k][:, f0:f0 + fsz],
                        start=(k == 0),
                        stop=(k == KC - 1),
                    )
                ao = dmap.tile([128, 512], F32, tag=f"ao{ci}_{f0}",
                               name=f"ao{ci}_{f0}")
                nc.vector.tensor_tensor(
                    ao[:sz, :fsz], ps_p[:sz, :fsz], bias_sb[:sz, f0:f0 + fsz],
                    op=mybir.AluOpType.add,
                )
                nc.sync.dma_start(
                    out_d[b, o:o + sz, f0:f0 + fsz], ao[:sz, :fsz]
                )

            pending_tail.extend([
                lambda: part_out_heads(range(0, 3)),
                lambda: part_out_heads(range(3, 6)),
                lambda: part_out_heads(range(6, 9)),
                lambda: part_out_heads(range(9, 12)),
                lambda: part_proj(0, N0, 0, 0, 512),
                lambda: part_proj(0, N0, 0, 512, 256),
                lambda: part_proj(N0, N1, 1, 0, 512),
                lambda: part_proj(N0, N1, 1, 512, 256),
            ])

        while pending_tail:
            pending_tail.pop(0)()

    nc.compile()
    return nc


def _prep_inputs(x, w_qkv, w_proj, b_proj, w_conv_l, w_conv_w):
    x = np.asarray(x, dtype=np.float32)
    w_qkv = np.asarray(w_qkv, dtype=np.float32).copy()
    w_proj = np.asarray(w_proj, dtype=np.float32)
    b_proj = np.asarray(b_proj, dtype=np.float32)
    w_conv_l = np.asarray(w_conv_l, dtype=np.float32)
    w_conv_w = np.asarray(w_conv_w, dtype=np.float32)

    # fold the attention scale into the q columns of w_qkv
    w_qkv[:, :C] *= SCALE
    wqkv_bf = w_qkv.astype(ml_dtypes.bfloat16)
    wproj_bf = w_proj.astype(ml_dtypes.bfloat16)
    bias_rep = np.ascontiguousarray(np.broadcast_to(b_proj, (128, C)))

    # x transposed per batch, bf16: [B, C, N]
    xT = np.ascontiguousarray(
        x.transpose(0, 2, 1).astype(ml_dtypes.bfloat16)
    )

    # conv_l as lhsT: WL[(h,j), (o,j)] = w_conv_l[o, h]; padded to 128 cols
    wl_b = np.zeros((128, 128), dtype=np.float32)
    # conv_w as moving operand: WW2[(o,j), (h,j)] = w_conv_w[h, o]
    ww2_b = np.zeros((128, HP), dtype=np.float32)
    idx = np.arange(GS)
    for a in range(H):
        for o in range(H):
            wl_b[a * GS + idx, o * GS + idx] = w_conv_l[o, a]
            ww2_b[a * GS + idx, o * GS + idx] = w_conv_w[o, a]
    wl_b = wl_b.astype(ml_dtypes.bfloat16)
    ww2_b = ww2_b.astype(ml_dtypes.bfloat16)
    idb = np.eye(128, dtype=ml_dtypes.bfloat16)

    in_maps = []
    for c in range(NCORES):
        in_maps.append({
            "xT": np.ascontiguousarray(xT[c * BPC:(c + 1) * BPC]),
            "w_qkv": wqkv_bf,
            "w_proj": wproj_bf,
            "bias": bias_rep,
            "wl": wl_b,
            "ww2": ww2_b,
            "idb": idb,
        })
    return in_maps


def _postprocess(outs):
    """Gather per-core results, normalize softmaxes, upcast to f32."""
    att = np.concatenate([o["attn_out"] for o in outs], axis=0)

    def unpack_map(a):
        # [B, HP=(h,j), G*N=(g,m)] -> [B, H, N, N] with n = g*GS + j
        a = a.reshape(B, H, GS, G, N).transpose(0, 1, 3, 2, 4)
        return a.reshape(B, H, NPAD, N)[:, :, :N, :]

    sc = unpack_map(
        np.concatenate([o["scores"] for o in outs], axis=0)
    ).astype(np.float32)

    pexp = unpack_map(np.concatenate([o["pexp"] for o in outs], axis=0))
    prcp = np.concatenate([o["prcp"] for o in outs], axis=0)  # [B, HP, G]
    n_idx = np.arange(N)
    prcp = prcp.reshape(B, H, GS, G)
    rcp = prcp[:, :, n_idx % GS, n_idx // GS]  # [B, H, N]
    pr = pexp.astype(np.float32) * rcp[:, :, :, None]

    # vexp: [B, H/2, 128, 4=(par,chunk), N] -> [B, H, N, N]
    vexp_r = np.concatenate([o["vexp"] for o in outs], axis=0)
    vexp_r = vexp_r.reshape(B, H // 2, 128, 2, 2, N).transpose(0, 1, 3, 2, 4, 5)
    vexp_r = vexp_r.reshape(B, H, 128, 2, N)
    vexp = np.concatenate(
        [vexp_r[:, :, :, 0, :], vexp_r[:, :, :N1, 1, :]], axis=2
    )
    vsum = np.concatenate([o["vsum"] for o in outs], axis=0)  # [B, 128, 2H]
    vsum = vsum.reshape(B, 128, H // 2, 2, 2).transpose(0, 1, 2, 3, 4)
    vsum = vsum.reshape(B, 128, H, 2)  # [..., h=(hp,par), chunk]
    vs = np.concatenate(
        [vsum[:, :, :, 0], vsum[:, :N1, :, 1]], axis=1
    ).transpose(0, 2, 1)  # [B, H, N]
    vm = vexp.astype(np.float32) / vs[:, :, :, None]

    att = np.ascontiguousarray(att, dtype=np.float32)
    return att, sc, np.ascontiguousarray(pr), np.ascontiguousarray(vm)


def _ensure_trace_support():
    """Install the antenv.axon_hooks NTFF shim missing from this image."""
    import sys
    import types
    try:
        import antenv.axon_hooks  # noqa: F401
        return
    except ImportError:
        pass
    import antenv
    from trn_agent_boot.trn_boot import _ntff_profile_via_ctypes
    hook = {"fn": _ntff_profile_via_ctypes("/opt/axon/libaxon_pjrt.so")}
    mod = types.ModuleType("antenv.axon_hooks")
    mod.get_axon_ntff_profile_hook = lambda: hook["fn"]
    mod.set_axon_ntff_profile_hook = lambda fn: hook.update(fn=fn)
    sys.modules["antenv.axon_hooks"] = mod
    antenv.axon_hooks = mod
    import concourse.bass_utils as bu
    bu.upload_artifacts = lambda tmpdir: f"local://{tmpdir}"


def _run(inputs, trace=False, trace_kwargs=None):
    if trace:
        _ensure_trace_support()
    if "nc" not in _CACHE:
        _CACHE["nc"] = _build_program()
    nc = _CACHE["nc"]
    in_maps = _prep_inputs(**inputs)
    res = run_bass_kernel_spmd(
        nc, in_maps, list(range(NCORES)), trace=trace,
        **({"trace_kwargs": trace_kwargs} if trace_kwargs else {}),
    )
    return _postprocess(res.results), res


def kernel(**inputs):
    (att, sc, pr, vm), _ = _run(inputs, trace=False)
    return att, sc, pr, vm


# revision 35
# speedup vs baseline: 1.0750x; 1.0250x over previous
"""Trainium2 Bass kernel for MiniAttention (sparse_attention variant).

Reference computation (per batch b):
  qkv = x @ w_qkv -> split q,k,v  [H=12 heads, N=197 tokens, D=64]
  value_map = softmax((v @ v^T) * scale)          [H,N,N]   (output 4)
  scores    = (q*scale) @ k^T                     [H,N,N]   (output 2)
  attn_l    = conv_l mixing over heads of scores
  probs     = softmax(attn_l)                     [H,N,N]   (output 3)
  attn_w    = conv_w mixing over heads of probs
  out       = attn_w @ v -> proj -> + bias        [N,C]     (output 1)

Sharding: pure data-parallel over batch B=32 across 8 NeuronCores (4 each).

Layout trick: the head-mixing 1x1 convs contract over H=12, too small for the
128x128 PE.  Attention maps live as tiles [120 partitions = (h in 12) x
(j in 10 tokens), m free], so conv_l is one K=120 matmul with a constant
block-structured matrix WL[(h,j),(o,j')] = w_conv_l[o,h] d_jj'.  The scores
matmul is emitted directly into this layout using block-diagonal lhsT tiles
holding q for two heads per 128-row contraction chunk.  conv_w is fused with
the transpose the attn_w @ v matmul needs: awT[m,(h,j)] = probs^T @ WW2 with
WW2[(o,j),(h,j')] = w_conv_w[h,o] d_jj' as the *moving* operand, so attn_w is
produced directly m-major.

Softmax normalization (the divide) is done on the host: the kernel ships
exp() maps in bf16 plus the per-row sums/reciprocals in f32; conv_w's use of
normalized probs is handled by scaling WW2's rows with the reciprocals
(one [120,120] tensor_scalar per group).
"""

import numpy as np
import ml_dtypes
from contextlib import ExitStack

import concourse.bass as bass
import concourse.mybir as mybir
import concourse.tile as tile
from concourse import bacc
from concourse.bass_utils import run_bass_kernel_spmd

F32 = mybir.dt.float32
BF16 = mybir.dt.bfloat16
AF = mybir.ActivationFunctionType

B, N, C, H = 32, 197, 768, 12
D = C // H            # 64
SCALE = D ** -0.5     # 0.125
NCORES = 8
BPC = B // NCORES     # 4 batches per core
GS = 10               # token-group size in the (h, j) partition layout
G = 20                # number of token groups (covers NPAD=200 >= N)
NPAD = G * GS         # 200
HP = H * GS           # 120 partitions used in map tiles
KC = C // 128         # 6 contraction chunks of 128
MC = 3 * C // 128     # 18 output chunks of qkv
QW = 256              # padded qkvT tile width (zeros beyond N)
N0 = 128              # first token chunk
N1 = N - N0           # 69

_CACHE = {}


def _build_program():
    """Build the (SPMD, value-independent) Bass program once."""
    nc = bacc.Bacc(
        "TRN2", target_bir_lowering=False, debug=False, num_devices=NCORES
    )

    # ---- DRAM I/O -------------------------------------------------------
    # x arrives pre-transposed and bf16 from the host: [BPC, C, N]
    xT_d = nc.dram_tensor("xT", [BPC, C, N], BF16, kind="ExternalInput")
    wqkv_d = nc.dram_tensor("w_qkv", [C, 3 * C], BF16, kind="ExternalInput")
    wproj_d = nc.dram_tensor("w_proj", [C, C], BF16, kind="ExternalInput")
    bias_d = nc.dram_tensor("bias", [128, C], F32, kind="ExternalInput")
    wl_d = nc.dram_tensor("wl", [128, 128], BF16, kind="ExternalInput")
    ww2_d = nc.dram_tensor("ww2", [128, HP], BF16, kind="ExternalInput")
    idb_d = nc.dram_tensor("idb", [128, 128], BF16, kind="ExternalInput")

    out_d = nc.dram_tensor("attn_out", [BPC, N, C], F32, kind="ExternalOutput")
    scores_d = nc.dram_tensor("scores", [BPC, HP, G * N], BF16,
                              kind="ExternalOutput")
    pexp_d = nc.dram_tensor("pexp", [BPC, HP, G * N], BF16,
                            kind="ExternalOutput")
    prcp_d = nc.dram_tensor("prcp", [BPC, HP, G], F32, kind="ExternalOutput")
    vexp_d = nc.dram_tensor("vexp", [BPC, H // 2, 128, 4, N], BF16, kind="ExternalOutput")
    vsum_d = nc.dram_tensor("vsum", [BPC, 128, 2 * H], F32, kind="ExternalOutput")

    with tile.TileContext(nc) as tc, ExitStack() as ctx:
        consts = ctx.enter_context(tc.tile_pool(name="consts", bufs=1))
        qkvp = ctx.enter_context(tc.tile_pool(name="qkvp", bufs=2))
        work = ctx.enter_context(tc.tile_pool(name="work", bufs=2))
        scp = ctx.enter_context(tc.tile_pool(name="scp", bufs=2))
        exw = ctx.enter_context(tc.tile_pool(name="exw", bufs=2))
        dmap = ctx.enter_context(tc.tile_pool(name="dmap", bufs=2))
        sums = ctx.enter_context(tc.tile_pool(name="sums", bufs=4))
        psA = ctx.enter_context(
            tc.tile_pool(name="psA", bufs=4, space=bass.MemorySpace.PSUM)
        )
        ps2 = ctx.enter_context(
            tc.tile_pool(name="ps2", bufs=2, space=bass.MemorySpace.PSUM)
        )

        # ---- constants (qkv weights first, 3-way queue split) ------------
        wqkv_sb = []
        for k in range(KC):
            t = consts.tile([128, 3 * C], BF16, tag=f"wqkv{k}", name=f"wqkv{k}")
            eng = nc.sync if k % 2 == 0 else nc.scalar
            eng.dma_start(t[:], wqkv_d[k * 128:(k + 1) * 128, :])
            wqkv_sb.append(t)
        idb_sb = consts.tile([128, 128], BF16, tag="idb", name="idb")
        nc.sync.dma_start(idb_sb[:], idb_d[:])
        wl_sb = consts.tile([128, 128], BF16, tag="wl", name="wl")
        nc.sync.dma_start(wl_sb[:], wl_d[:])
        ww2_sb = consts.tile([128, HP], BF16, tag="ww2", name="ww2")
        nc.sync.dma_start(ww2_sb[:], ww2_d[:])
        wproj_sb = []
        for k in range(KC):
            t = consts.tile([128, C], BF16, tag=f"wproj{k}", name=f"wproj{k}")
            eng = nc.scalar if k % 2 == 0 else nc.sync
            eng.dma_start(t[:], wproj_d[k * 128:(k + 1) * 128, :])
            wproj_sb.append(t)
        bias_sb = consts.tile([128, C], F32, tag="bias", name="bias")
        nc.scalar.dma_start(bias_sb[:], bias_d[:])
        zbias = consts.tile([128, 1], F32, tag="zbias", name="zbias")
        nc.gpsimd.memset(zbias[:], 0.0)

        # block-diagonal q lhsT tiles: allocated once, zeroed once; the
        # nonzero q blocks are fully overwritten every batch.  Per-group
        # stride is 128 columns (120 used + 8 zero) so the weight loads see
        # full 128-column tiles (FWL); psum rows 120:128 are junk.
        BDW = 128
        bd = []
        for k in range(KC):
            t = consts.tile([128, G * BDW], BF16, tag=f"bd{k}", name=f"bdt{k}")
            nc.gpsimd.memset(t[:], 0.0)
            bd.append(t)

        qkvT = {}   # batch -> list of 18 [128, NPAD] bf16 tile views

        def stage_qkv(b0):
            """Compute qkvT for batches b0, b0+1 (paired rhs)."""
            xT = []
            for k in range(KC):
                t = qkvp.tile([128, 2 * N], BF16, tag=f"xT{k}", name=f"xT{k}")
                for bi in range(2):
                    nc.sync.dma_start(
                        t[:, bi * N:(bi + 1) * N],
                        xT_d[b0 + bi, k * 128:(k + 1) * 128, :],
                    )
                xT.append(t)
            for mi in range(MC):
                pq = psA.tile([128, 2 * N], F32, tag="ps", name="ps_qkv")
                for k in range(KC):
                    nc.tensor.matmul(
                        pq[:],
                        wqkv_sb[k][:, mi * 128:(mi + 1) * 128],
                        xT[k][:],
                        start=(k == 0),
                        stop=(k == KC - 1),
                    )
                t = qkvp.tile([128, 2 * QW], BF16, tag=f"qkvT{mi}",
                              name=f"qkvT{mi}")
                t3 = t[:].rearrange("p (b n) -> p b n", n=QW)
                ceng = nc.vector if mi % 2 == 0 else nc.scalar
                if ceng is nc.vector:
                    nc.vector.tensor_copy(t3[:, :, :N], pq[:].rearrange(
                        "p (b n) -> p b n", n=N))
                else:
                    nc.scalar.copy(t3[:, :, :N], pq[:].rearrange(
                        "p (b n) -> p b n", n=N))
                nc.gpsimd.memset(t3[:, :, N:QW], 0.0)
                for bi in range(2):
                    qkvT.setdefault(b0 + bi, [None] * MC)[mi] = t[
                        :, bi * QW:bi * QW + QW
                    ]

        pending_tail = []
        for b in range(BPC):
            if b == 0:
                stage_qkv(b)
            qk = qkvT[b]

            # ---- fill block-diagonal q lhsT tiles for this batch ---------
            for k in range(KC):
                dst = bd[k][:].rearrange("p (g t) -> p g t", t=BDW)
                src = qk[k][:, :NPAD].rearrange("p (g j) -> p g j", j=GS)
                h0, h1 = 2 * k, 2 * k + 1
                nc.gpsimd.tensor_copy(
                    dst[0:64, :, h0 * GS:(h0 + 1) * GS], src[0:64, :, :]
                )
                nc.gpsimd.tensor_copy(
                    dst[64:128, :, h1 * GS:(h1 + 1) * GS], src[64:128, :, :]
                )

            # ---- v^T -> v (token-major) packed tiles ----------------------
            v0 = work.tile([128, H * D], BF16, tag="v0", name="v0")
            v1 = work.tile([128, H * D], BF16, tag="v1", name="v1")
            for h in range(H):
                ch = 2 * KC + h // 2
                r0 = (h % 2) * 64
                for (o, sz, vt) in ((0, N0, v0), (N0, N1, v1)):
                    pt = psA.tile([128, 128], BF16, tag="ps", name="ps_trb")
                    nc.tensor.transpose(
                        pt[:sz, :D],
                        qk[ch][r0:r0 + 64, o:o + sz],
                        idb_sb[r0:r0 + 64, r0:r0 + 64],
                    )
                    nc.vector.tensor_copy(
                        vt[:sz, h * D:(h + 1) * D], pt[:sz, :D]
                    )

            awT0 = work.tile([128, G * HP], BF16, tag="awT0", name="awT0")
            awT1 = work.tile([N1, G * HP], BF16, tag="awT1", name="awT1")

            # ---- scores (all groups first: dense PE run) ------------------
            sc_all = scp.tile([128, G * N], BF16, tag="sc_all", name="sc_all")
            sc_tiles = [sc_all[:, g * N:(g + 1) * N] for g in range(G)]
            for g in range(G):
                ps_sc = psA.tile([128, N], F32, tag="ps", name="ps_sc")
                for k in range(KC):
                    nc.tensor.matmul(
                        ps_sc[:],
                        bd[k][:, g * BDW:(g + 1) * BDW],
                        qk[KC + k][:, :N],
                        start=(k == 0),
                        stop=(k == KC - 1),
                    )
                eng = nc.vector if g % 2 == 0 else nc.scalar
                if eng is nc.vector:
                    nc.vector.tensor_copy(sc_tiles[g][:], ps_sc[:])
                else:
                    nc.scalar.copy(sc_tiles[g][:], ps_sc[:])
            nc.sync.dma_start(scores_d[b], sc_all[:HP, :])

            # ---- value map emitter -----------------------------------------
            vs_pack = dmap.tile([128, 2 * H], F32, tag="vs", name="vs")

            def emit_vmap_pair(hp, b=b, qk=qk, vs_pack=vs_pack):
                ch = 2 * KC + hp
                ps_v = ps2.tile([128, 1024], F32, tag="ps2", name="ps_v")
                for par in range(2):
                    r0 = par * 64
                    c0 = par * 512
                    nc.tensor.matmul(
                        ps_v[:, c0:c0 + N],
                        qk[ch][r0:r0 + 64, 0:N0],
                        qk[ch][r0:r0 + 64, :N],
                        start=True, stop=True,
                    )
                    nc.tensor.matmul(
                        ps_v[:, c0 + N:c0 + 2 * N],
                        qk[ch][r0:r0 + 64, N0:N0 + 128],
                        qk[ch][r0:r0 + 64, :N],
                        start=True, stop=True,
                    )
                pin = ps_v[:].rearrange("p (a q) -> p a q", a=2)[:, :, :2 * N]
                pin = pin.rearrange("p a (t m) -> p a t m", m=N)
                vex = exw.tile([128, 4 * N], BF16, tag="vex", name="vex")
                nc.scalar.activation(
                    vex[:].rearrange("p (a t m) -> p a t m", a=2, m=N),
                    pin, AF.Exp, bias=zbias[:], scale=float(SCALE),
                )
                nc.vector.tensor_reduce(
                    out=vs_pack[:, 4 * hp:4 * hp + 4],
                    in_=vex[:].rearrange("p (t m) -> p t m", m=N),
                    op=mybir.AluOpType.add,
                    axis=mybir.AxisListType.X,
                )
                nc.sync.dma_start(
                    vexp_d[b, hp],
                    vex[:].rearrange("p (t m) -> p t m", m=N),
                )

            # ---- pass A: conv_l -> exp -> sums -> scaled WW2 ---------------
            # (vmap heads interleaved; results packed so pass B can run
            # densely after the deferred tail of the previous batch)
            rcp_pack = dmap.tile([128, G], F32, tag="rcp_pack", name="rcp_pack")
            nc.gpsimd.memset(rcp_pack[96:, :], 0.0)
            ex_all = exw.tile([128, G * N], BF16, tag="ex_all", name="ex_all")
            ww2s_all = exw.tile([128, G * HP], BF16, tag="ww2s_all",
                                name="ww2s_all")
            for t4 in range(G // 4):
                g0 = 4 * t4
                ps_al = ps2.tile([128, 1024], F32, tag="ps2", name="ps_al")
                for a in range(2):
                    nc.tensor.matmul(
                        ps_al[:, 512 * a:512 * a + 2 * N], wl_sb[:],
                        sc_all[:, (g0 + 2 * a) * N:(g0 + 2 * a + 2) * N],
                        start=True, stop=True,
                    )
                if t4 < G // 4 - 1:
                    emit_vmap_pair(t4)
                else:
                    emit_vmap_pair(4)
                    emit_vmap_pair(5)
                ex = ex_all[:, g0 * N:(g0 + 4) * N]
                pin = ps_al[:].rearrange("p (a q) -> p a q", a=2)[:, :, :2 * N]
                pin = pin.rearrange("p a (u m) -> p a u m", m=N)
                nc.scalar.activation(
                    ex.rearrange("p (a u m) -> p a u m", a=2, m=N),
                    pin, AF.Exp, bias=zbias[:, :],
                )
                sm4 = sums.tile([HP, 4], F32, tag="sm4", name="sm4")
                nc.vector.tensor_reduce(
                    out=sm4[:],
                    in_=ex[:HP].rearrange("p (u m) -> p u m", m=N),
                    op=mybir.AluOpType.add,
                    axis=mybir.AxisListType.X,
                )
                nc.vector.reciprocal(rcp_pack[:HP, g0:g0 + 4], sm4[:])
                for g in range(g0, g0 + 4):
                    nc.vector.tensor_scalar_mul(
                        ww2s_all[:, g * HP:(g + 1) * HP], ww2_sb[:],
                        rcp_pack[:, g:g + 1],
                    )  # pad rows: 0 * rcp(0) = 0
            nc.scalar.dma_start(pexp_d[b], ex_all[:HP, :])
            nc.sync.dma_start(prcp_d[b], rcp_pack[:HP, :])

            # ---- prefetch the next pair's qkv stage ------------------------
            if b % 2 == 1 and b + 1 < BPC:
                stage_qkv(b + 1)

            # ---- deferred tail of the previous batch -----------------------
            while pending_tail:
                pending_tail.pop(0)()

            # ---- pass B: conv_w^T (dense PE run) ---------------------------
            for g in range(G):
                for (o, sz, awt) in ((0, N0, awT0), (N0, N1, awT1)):
                    ps_aw = psA.tile([128, HP], F32, tag="ps", name="ps_aw")
                    nc.tensor.matmul(
                        ps_aw[:sz, :], ex_all[:, g * N + o:g * N + o + sz],
                        ww2s_all[:, g * HP:(g + 1) * HP],
                        start=True, stop=True,
                    )
                    if g % 5 < 3:
                        nc.vector.tensor_copy(
                            awt[:sz, g * HP:(g + 1) * HP], ps_aw[:sz, :]
                        )
                    else:
                        nc.scalar.copy(
                            awt[:sz, g * HP:(g + 1) * HP], ps_aw[:sz, :]
                        )
            nc.sync.dma_start(vsum_d[b, :, :], vs_pack[:])

            # ---- tail parts (deferred into the next batch's conv loop) ---
            pj_box = []

            def part_out_heads(hs, b=b, awT0=awT0, awT1=awT1, v0=v0, v1=v1):
                if not pj_box:
                    for cch in range(KC):
                        t = work.tile([128, N], BF16, tag=f"pj{cch}",
                                      name=f"pj{cch}")
                        pj_box.append(t)
                for h in hs:
                    ps_o = psA.tile([64, NPAD], F32, tag="ps", name="ps_o")
                    rhs0 = awT0[:].rearrange("p (g t) -> p g t", t=HP)
                    rhs1 = awT1[:].rearrange("p (g t) -> p g t", t=HP)
                    nc.tensor.matmul(
                        ps_o[:],
                        v0[:, h * D:(h + 1) * D],
                        rhs0[:, :, h * GS:(h + 1) * GS],
                        start=True,
                        stop=False,
                    )
                    nc.tensor.matmul(
                        ps_o[:],
                        v1[:N1, h * D:(h + 1) * D],
                        rhs1[:N1, :, h * GS:(h + 1) * GS],
                        start=False,
                        stop=True,
                    )
                    if h % 2 == 0:
                        nc.scalar.copy(
                            pj_box[h // 2][:64, :], ps_o[:, :N],
                        )
                    else:
                        nc.vector.tensor_copy(
                            pj_box[h // 2][64:128, :], ps_o[:, :N],
                        )

            def part_proj(o, sz, ci, f0, fsz, b=b):
                ps_p = psA.tile([128, 512], F32, tag="ps", name="ps_p")
                for k in range(KC):
                    nc.tensor.matmul(
                        ps_p[:sz, :fsz],
                        pj_box[k][:, o:o + sz],
                        wproj_sb[k][:, f0:f0 + fsz],
                        start=(k == 0),
                        stop=(k == KC - 1),
                    )
                ao = dmap.tile([128, 512], F32, tag=f"ao{ci}_{f0}",
                               name=f"ao{ci}_{f0}")
                nc.vector.tensor_tensor(
                    ao[:sz, :fsz], ps_p[:sz, :fsz], bias_sb[:sz, f0:f0 + fsz],
                    op=mybir.AluOpType.add,
                )
                nc.sync.dma_start(
                    out_d[b, o:o + sz, f0:f0 + fsz], ao[:sz, :fsz]
                )

            pending_tail.extend([
                lambda: part_out_heads(range(0, 3)),
                lambda: part_out_heads(range(3, 6)),
                lambda: part_out_heads(range(6, 9)),
                lambda: part_out_heads(range(9, 12)),
                lambda: part_proj(0, N0, 0, 0, 512),
                lambda: part_proj(0, N0, 0, 512, 256),
                lambda: part_proj(N0, N1, 1, 0, 512),
                lambda: part_proj(N0, N1, 1, 512, 256),
            ])

        while pending_tail:
            pending_tail.pop(0)()

    nc.compile()
    return nc


def _prep_inputs(x, w_qkv, w_proj, b_proj, w_conv_l, w_conv_w):
    x = np.asarray(x, dtype=np.float32)
    w_qkv = np.asarray(w_qkv, dtype=np.float32).copy()
    w_proj = np.asarray(w_proj, dtype=np.float32)
    b_proj = np.asarray(b_proj, dtype=np.float32)
    w_conv_l = np.asarray(w_conv_l, dtype=np.float32)
    w_conv_w = np.asarray(w_conv_w, dtype=np.float32)

    # fold the attention scale into the q columns of w_qkv
    w_qkv[:, :C] *= SCALE
    wqkv_bf = w_qkv.astype(ml_dtypes.bfloat16)
    wproj_bf = w_proj.astype(ml_dtypes.bfloat16)
    bias_rep = np.ascontiguousarray(np.broadcast_to(b_proj, (128, C)))

    # x transposed per batch, bf16: [B, C, N]
    xT = np.ascontiguousarray(
        x.transpose(0, 2, 1).astype(ml_dtypes.bfloat16)
    )

    # conv_l as lhsT: WL[(h,j), (o,j)] = w_conv_l[o, h]; padded to 128 cols
    wl_b = np.zeros((128, 128), dtype=np.float32)
    # conv_w as moving operand: WW2[(o,j), (h,j)] = w_conv_w[h, o]
    ww2_b = np.zeros((128, HP), dtype=np.float32)
    idx = np.arange(GS)
    for a in range(H):
        for o in range(H):
            wl_b[a * GS + idx, o * GS + idx] = w_conv_l[o, a]
            ww2_b[a * GS + idx, o * GS + idx] = w_conv_w[o, a]
    wl_b = wl_b.astype(ml_dtypes.bfloat16)
    ww2_b = ww2_b.astype(ml_dtypes.bfloat16)
    idb = np.eye(128, dtype=ml_dtypes.bfloat16)

    in_maps = []
    for c in range(NCORES):
        in_maps.append({
            "xT": np.ascontiguousarray(xT[c * BPC:(c + 1) * BPC]),
            "w_qkv": wqkv_bf,
            "w_proj": wproj_bf,
            "bias": bias_rep,
            "wl": wl_b,
            "ww2": ww2_b,
            "idb": idb,
        })
    return in_maps


def _postprocess(outs):
    """Gather per-core results, normalize softmaxes, upcast to f32."""
    att = np.concatenate([o["attn_out"] for o in outs], axis=0)

    def unpack_map(a):
        # [B, HP=(h,j), G*N=(g,m)] -> [B, H, N, N] with n = g*GS + j
        a = a.reshape(B, H, GS, G, N).transpose(0, 1, 3, 2, 4)
        return a.reshape(B, H, NPAD, N)[:, :, :N, :]

    sc = unpack_map(
        np.concatenate([o["scores"] for o in outs], axis=0)
    ).astype(np.float32)

    pexp = unpack_map(np.concatenate([o["pexp"] for o in outs], axis=0))
    prcp = np.concatenate([o["prcp"] for o in outs], axis=0)  # [B, HP, G]
    n_idx = np.arange(N)
    prcp = prcp.reshape(B, H, GS, G)
    rcp = prcp[:, :, n_idx % GS, n_idx // GS]  # [B, H, N]
    pr = pexp.astype(np.float32) * rcp[:, :, :, None]

    # vexp: [B, H/2, 128, 4=(par,chunk), N] -> [B, H, N, N]
    vexp_r = np.concatenate([o["vexp"] for o in outs], axis=0)
    vexp_r = vexp_r.reshape(B, H // 2, 128, 2, 2, N).transpose(0, 1, 3, 2, 4, 5)
    vexp_r = vexp_r.reshape(B, H, 128, 2, N)
    vexp = np.concatenate(
        [vexp_r[:, :, :, 0, :], vexp_r[:, :, :N1, 1, :]], axis=2
    )
    vsum = np.concatenate([o["vsum"] for o in outs], axis=0)  # [B, 128, 2H]
    vsum = vsum.reshape(B, 128, H // 2, 2, 2).transpose(0, 1, 2, 3, 4)
    vsum = vsum.reshape(B, 128, H, 2)  # [..., h=(hp,par), chunk]
    vs = np.concatenate(
        [vsum[:, :, :, 0], vsum[:, :N1, :, 1]], axis=1
    ).transpose(0, 2, 1)  # [B, H, N]
    vm = vexp.astype(np.float32) / vs[:, :, :, None]

    att = np.ascontiguousarray(att, dtype=np.float32)
    return att, sc, np.ascontiguousarray(pr), np.ascontiguousarray(vm)


def _ensure_trace_support():
    """Install the antenv.axon_hooks NTFF shim missing from this image."""
    import sys
    import types
    try:
        import antenv.axon_hooks  # noqa: F401
        return
    except ImportError:
        pass
    import antenv
    from trn_agent_boot.trn_boot import _ntff_profile_via_ctypes
    hook = {"fn": _ntff_profile_via_ctypes("/opt/axon/libaxon_pjrt.so")}
    mod = types.ModuleType("antenv.axon_hooks")
    mod.get_axon_ntff_profile_hook = lambda: hook["fn"]
    mod.set_axon_ntff_profile_hook = lambda fn: hook.update(fn=fn)
    sys.modules["antenv.axon_hooks"] = mod
    antenv.axon_hooks = mod
    import concourse.bass_utils as bu
    bu.upload_artifacts = lambda tmpdir: f"local://{tmpdir}"


def _run(inputs, trace=False, trace_kwargs=None):
    if trace:
        _ensure_trace_support()
    if "nc" not in _CACHE:
        _CACHE["nc"] = _build_program()
    nc = _CACHE["nc"]
    in_maps = _prep_inputs(**inputs)
    res = run_bass_kernel_spmd(
        nc, in_maps, list(range(NCORES)), trace=trace,
        **({"trace_kwargs": trace_kwargs} if trace_kwargs else {}),
    )
    return _postprocess(res.results), res


def kernel(**inputs):
    (att, sc, pr, vm), _ = _run(inputs, trace=False)
    return att, sc, pr, vm
